# revision 13
# baseline (speedup 1.0000x reference)
"""Trainium2 Bass kernel for nn_Attention (B=2, S=4096, E=768, H=12, D=64).

Sharding: 24 (batch, head) units over 8 cores -> 3 heads per core, one batch
per 4-core group. Each core computes QKV projections for its 3 heads, full
attention (scoresT layout [j, i]; softmax across partitions via a
ones-augmented V matmul), and its partial out-projection [E, S]. The host
sums the 4 partials per batch and adds the fused output bias.

Math notes:
 - k bias dropped (softmax is shift-invariant along the key axis).
 - q bias and the 1/sqrt(D) scaling folded into the q weights/bias.
 - v bias folded into the output bias on host (sum_j softmax = 1).
 - All matmul operands are fp16 (accumulation fp32 in PSUM). fp16 keeps
   LDWEIGHTS off the critical path (pull-ahead + FWL) and enables
   concurrent row-tiled score matmuls (two heads, or two key tiles of the
   third head via duplicated hi/lo q/k copies).

Schedule: ScalarE (exp over all S^2 scores) is the bottleneck engine, so
ic=0's attention is interleaved with the QKV projections to start exp work
within the first few microseconds; afterwards the per-i-chunk pipeline
keeps ScalarE 100% busy (measured) while PE runs scores/AV/out-proj
underneath it.
"""

import numpy as np

B = 2
S = 4096
E = 768
NHEADS = 12
D = 64
SCALING = float(D) ** -0.5
N_CORES = 8
HPC = 3  # heads per core
CORES_PER_BATCH = 4

_PROGRAM_CACHE = {}


def _build_program(s=S):
    import concourse.mybir as mybir
    import concourse.tile as tile
    from concourse import bacc

    f32 = mybir.dt.float32
    f16 = mybir.dt.float16
    Exp = mybir.ActivationFunctionType.Exp

    IC = 512  # i-chunk (query block, PSUM free dim)
    NIC = s // IC
    NJT = s // 128  # key tiles of 128
    NE = E // 128  # contraction tiles for the projections

    nc = bacc.Bacc(
        "TRN2", target_bir_lowering=False, debug=False, num_devices=N_CORES
    )

    xT_d = nc.dram_tensor("xT", [E, s], f16, kind="ExternalInput").ap()
    wqk_d = nc.dram_tensor("wqk", [E, 256], f16, kind="ExternalInput").ap()
    w2_d = nc.dram_tensor("w2", [E, 256], f16, kind="ExternalInput").ap()
    wv_d = nc.dram_tensor("wv", [E, 256], f16, kind="ExternalInput").ap()
    wout_d = nc.dram_tensor("wout", [D, HPC * E], f16, kind="ExternalInput").ap()
    bq_d = nc.dram_tensor("bq", [256, 1], f32, kind="ExternalInput").ap()
    out_d = nc.dram_tensor("out_part", [E, s], f32, kind="ExternalOutput").ap()

    with tile.TileContext(nc) as tc:
        with (
            tc.tile_pool(name="consts", bufs=1) as consts,
            tc.tile_pool(name="persist", bufs=1) as persist,
        ):
            # ---- weights ----
            def load_w(dram, cols, tag):
                t = consts.tile([128, NE * cols], f16, tag=tag)
                nc.sync.dma_start(
                    t[:].rearrange("p (e c) -> p e c", e=NE),
                    dram.rearrange("(e p) c -> p e c", p=128),
                )
                return t

            wqk_sb = load_w(wqk_d, 256, "wqk")
            w2_sb = load_w(w2_d, 256, "w2")
            wv_sb = load_w(wv_d, 256, "wv")
            wout_sb = consts.tile([D, HPC * E], f16, tag="wout")
            nc.sync.dma_start(wout_sb[:], wout_d)
            bqp_sb = consts.tile([128, 1], f32, tag="bqp")
            nc.sync.dma_start(bqp_sb[:], bq_d[0:128, :])
            bq2_sb = consts.tile([128, 1], f32, tag="bq2")
            nc.sync.dma_start(bq2_sb[:], bq_d[128:256, :])

            # ---- persistent activations ----
            # pair: h0 on partitions 0:64, h1 on 64:128
            # h2: duplicated on both partition halves (row-tiled dual stream)
            qTp = persist.tile([128, s], f16, tag="qTp")
            kTp = persist.tile([128, s], f16, tag="kTp")
            qT2 = persist.tile([128, s], f16, tag="qT2")
            kT2 = persist.tile([128, s], f16, tag="kT2")
            # v in natural [j, d] layout, 65-wide slots (col 64 = ones)
            v_aug = persist.tile([128, HPC * NJT * 65], f16, tag="vaug")
            vview = v_aug[:].rearrange("p (h j c) -> p h j c", h=HPC, c=65)
            ones_src = consts.tile([128, 1], f32, tag="ones")
            nc.vector.memset(ones_src[:], 1.0)
            nc.vector.tensor_copy(
                v_aug[:].rearrange("p (t c) -> p t c", c=65)[:, :, 64:65],
                ones_src[:, None, :].broadcast_to([128, HPC * NJT, 1]),
            )

            class Dual:
                """Slice-granular dual-row-tiled scores -> grouped exp -> AV,
                skewed by one exp group so PE never stalls on a fresh exp.

                One slice = one [128, 512] scores matmul on one partition
                half (row-tiled, so adjacent slices run concurrently on the
                PE). G slices share one PSUM tile and one exp op.
                """

                def __init__(self, sc_pool, e_pool, ic, kT, qT, G,
                             jt_of, rh_of, av_of, head_of):
                    self.sc_pool, self.e_pool = sc_pool, e_pool
                    self.isl = slice(ic * IC, (ic + 1) * IC)
                    self.kT, self.qT, self.G = kT, qT, G
                    self.jt_of, self.rh_of = jt_of, rh_of
                    self.av_of, self.head_of = av_of, head_of
                    self.s = 0
                    self.cur = None
                    self.pending = None  # (et, base_slice, count)

                def _avs(self, et, base, count):
                    for k in range(count):
                        sl = base + k
                        jt = self.jt_of(sl)
                        nc.tensor.matmul(
                            self.av_of(sl)[:],
                            vview[:, self.head_of(sl), jt, :],
                            et[:, k * 512 : (k + 1) * 512],
                            start=(jt == 0),
                            stop=(jt == NJT - 1),
                        )

                def _fire(self, count):
                    base = self.s - count
                    et = self.e_pool.tile([128, 512 * self.G], f16, tag="e")
                    nc.scalar.activation(
                        et[:, : 512 * count], self.cur[:, : 512 * count], Exp
                    )
                    if self.pending is not None:
                        self._avs(*self.pending)
                    self.pending = (et, base, count)
                    self.cur = None

                def emit(self):
                    pos = self.s % self.G
                    if pos == 0:
                        self.cur = self.sc_pool.tile(
                            [128, 512 * self.G], f32, tag="sc"
                        )
                    jt = self.jt_of(self.s)
                    rh = self.rh_of(self.s)
                    plo = rh * 64
                    nc.tensor.matmul(
                        self.cur[:, pos * 512 : (pos + 1) * 512],
                        self.kT[plo : plo + 64, jt * 128 : (jt + 1) * 128],
                        self.qT[plo : plo + 64, self.isl],
                        start=True,
                        stop=True,
                        tile_position=(plo, 0),
                    )
                    self.s += 1
                    if self.s % self.G == 0:
                        self._fire(self.G)

                def finish(self):
                    if self.s % self.G != 0:
                        self._fire(self.s % self.G)
                    if self.pending is not None:
                        self._avs(*self.pending)
                        self.pending = None

            with (
                tc.tile_pool(name="xt", bufs=NIC * NE) as xt_pool,
                tc.tile_pool(name="e_sb", bufs=4) as e_pool,
                tc.tile_pool(name="outt", bufs=2) as outT_pool,
                tc.tile_pool(name="bcast", bufs=2) as bcast_pool,
                tc.tile_pool(name="recip", bufs=2) as recip_pool,
                tc.tile_pool(name="op_out", bufs=3) as op_out_pool,
            ):
                def normalize(av, outT, h):
                    srow = recip_pool.tile([1, IC], f32, tag="srow")
                    nc.vector.tensor_copy(srow[:], av[64:65, :])
                    rc = recip_pool.tile([1, IC], f32, tag="recip")
                    nc.vector.reciprocal_approx_fast(rc[:], srow[:])
                    bc = bcast_pool.tile([64, IC], f32, tag="bcast")
                    nc.gpsimd.partition_broadcast(bc[:], rc[:], channels=64)
                    nc.vector.tensor_mul(
                        outT[:, h * IC : (h + 1) * IC], av[0:64, :], bc[:]
                    )

                # All xT loads up front (tiles stay resident; the DMA
                # engines run ahead of compute)
                xts = {}
                for cg in range(NIC):
                    for e in range(NE):
                        t = xt_pool.tile([128, IC], f16, tag="xt")
                        nc.sync.dma_start(
                            t[:],
                            xT_d[e * 128 : (e + 1) * 128,
                                 cg * IC : (cg + 1) * IC],
                        )
                        xts[(cg, e)] = t

                # ---- interleaved projections + ic=0 attention ----
                outT0 = outT_pool.tile([64, HPC * IC], f16, tag="outt")
                with (
                    tc.tile_pool(name="acc_i", bufs=4, space="PSUM") as acc_i,
                    tc.tile_pool(name="proj_ps", bufs=2, space="PSUM") as proj_ps,
                    tc.tile_pool(name="sc_i", bufs=1, space="PSUM") as sc_i,
                ):
                    av0 = acc_i.tile([65, IC], f32, tag="acc")
                    av1 = acc_i.tile([65, IC], f32, tag="acc")
                    av2 = acc_i.tile([65, IC], f32, tag="acc")
                    avs = (av0, av1)
                    pairE = Dual(sc_i, e_pool, 0, kTp, qTp, 2,
                                 jt_of=lambda s: s // 2,
                                 rh_of=lambda s: s % 2,
                                 av_of=lambda s: avs[s % 2],
                                 head_of=lambda s: s % 2)
                    h2E = Dual(sc_i, e_pool, 0, kT2, qT2, 2,
                               jt_of=lambda s: s,
                               rh_of=lambda s: s % 2,
                               av_of=lambda s: av2,
                               head_of=lambda s: 2)
                    for cg in range(NIC):
                        cs = slice(cg * IC, (cg + 1) * IC)
                        # q/k pair projections, then the third head's
                        # (two psum groups at a time; bufs=2 rotation)
                        for wsb, qdst, kdst, bias in (
                            (wqk_sb, qTp, kTp, bqp_sb),
                            (w2_sb, qT2, kT2, bq2_sb),
                        ):
                            qps = proj_ps.tile([128, IC], f32, tag="proj")
                            kps = proj_ps.tile([128, IC], f32, tag="proj")
                            for e in range(NE):
                                st = dict(start=(e == 0), stop=(e == NE - 1))
                                c0 = e * 256
                                xe = xts[(cg, e)][:]
                                nc.tensor.matmul(
                                    qps[:], wsb[:, c0 : c0 + 128], xe, **st
                                )
                                nc.tensor.matmul(
                                    kps[:], wsb[:, c0 + 128 : c0 + 256], xe, **st
                                )
                            nc.vector.tensor_scalar_add(
                                qdst[:, cs], qps[:], bias[:]
                            )
                            nc.vector.tensor_copy(kdst[:, cs], kps[:])
                        # v projections + ic0 attention slices, interleaved
                        for l in range(4):
                            jt = cg * 4 + l
                            vp = acc_i.tile([128, 256], f32, tag="acc")
                            for e in range(NE):
                                nc.tensor.matmul(
                                    vp[:],
                                    xts[(cg, e)][:, l * 128 : (l + 1) * 128],
                                    wv_sb[:, e * 256 : (e + 1) * 256],
                                    start=(e == 0),
                                    stop=(e == NE - 1),
                                )
                            nc.vector.tensor_copy(
                                vview[:, :, jt, 0:64],
                                vp[:, 0:192].rearrange("p (h d) -> p h d", h=HPC),
                            )
                            pairE.emit()
                            pairE.emit()
                            # with a single scores slot a group must fully
                            # fire before the other emitter allocates it
                            if l % 2 == 0:
                                h2E.emit()
                                h2E.emit()
                    pairE.finish()
                    h2E.finish()
                    normalize(av0, outT0, 0)
                    normalize(av1, outT0, 1)
                    normalize(av2, outT0, 2)
                pending = (outT0, 0)

                # ---- steady state: i-chunks 1..NIC-1 ----
                with (
                    tc.tile_pool(name="acc_s", bufs=2, space="PSUM") as acc_s,
                    tc.tile_pool(name="sc_ps", bufs=2, space="PSUM") as sc_s,
                ):
                    def outproj(outT, ic):
                        isl = slice(ic * IC, (ic + 1) * IC)
                        for fb in range(E // 128):
                            op = acc_s.tile([128, IC], f32, tag="acc")
                            for h in range(HPC):
                                nc.tensor.matmul(
                                    op[:],
                                    wout_sb[
                                        :,
                                        h * E + fb * 128 : h * E + (fb + 1) * 128,
                                    ],
                                    outT[:, h * IC : (h + 1) * IC],
                                    start=(h == 0),
                                    stop=(h == HPC - 1),
                                )
                            ob = op_out_pool.tile([128, IC], f32, tag="ob")
                            nc.vector.tensor_copy(ob[:], op[:])
                            nc.sync.dma_start(
                                out_d[fb * 128 : (fb + 1) * 128, isl], ob[:]
                            )

                    for ic in range(1, NIC):
                        outT = outT_pool.tile([64, HPC * IC], f16, tag="outt")
                        a0 = acc_s.tile([65, IC], f32, tag="acc")
                        a1 = acc_s.tile([65, IC], f32, tag="acc")
                        aa = (a0, a1)
                        d = Dual(sc_s, e_pool, ic, kTp, qTp, 3,
                                 jt_of=lambda s: s // 2,
                                 rh_of=lambda s: s % 2,
                                 av_of=lambda s, aa=aa: aa[s % 2],
                                 head_of=lambda s: s % 2)
                        for _ in range(2 * NJT):
                            d.emit()
                        d.finish()
                        normalize(a0, outT, 0)
                        normalize(a1, outT, 1)
                        outproj(*pending)
                        a2 = acc_s.tile([65, IC], f32, tag="acc")
                        d2 = Dual(sc_s, e_pool, ic, kT2, qT2, 3,
                                  jt_of=lambda s: s,
                                  rh_of=lambda s: s % 2,
                                  av_of=lambda s, a2=a2: a2,
                                  head_of=lambda s: 2)
                        for _ in range(NJT):
                            d2.emit()
                        d2.finish()
                        normalize(a2, outT, 2)
                        pending = (outT, ic)
                    outproj(*pending)

    nc.compile()
    return nc


def _core_inputs(x, in_proj_weight, in_proj_bias, out_proj_weight, core):
    """Host-side slicing for one core."""
    b = core // CORES_PER_BATCH
    h0 = HPC * (core % CORES_PER_BATCH)
    heads = [h0, h0 + 1, h0 + 2]

    wq = in_proj_weight[0:E]  # [E(out), E(in)]
    wk = in_proj_weight[E : 2 * E]
    wv = in_proj_weight[2 * E : 3 * E]
    bq_full = in_proj_bias[0:E]

    def head_wT(w, h):  # -> [E(in), D] = W_h.T
        return w[h * D : (h + 1) * D, :].T

    xT = np.ascontiguousarray(x[b].T).astype(np.float16)

    wqk = np.concatenate(
        [
            head_wT(wq, heads[0]) * SCALING,
            head_wT(wq, heads[1]) * SCALING,
            head_wT(wk, heads[0]),
            head_wT(wk, heads[1]),
        ],
        axis=1,
    ).astype(np.float16)
    wq2 = head_wT(wq, heads[2]) * SCALING
    wk2 = head_wT(wk, heads[2])
    w2 = np.concatenate([wq2, wq2, wk2, wk2], axis=1).astype(np.float16)
    wv_arr = np.concatenate(
        [head_wT(wv, h) for h in heads] + [np.zeros((E, D), np.float32)], axis=1
    ).astype(np.float16)
    wout = np.concatenate(
        [out_proj_weight[:, h * D : (h + 1) * D].T for h in heads], axis=1
    ).astype(np.float16)
    bq2 = bq_full[heads[2] * D : (heads[2] + 1) * D] * SCALING
    bq = np.concatenate(
        [
            bq_full[heads[0] * D : (heads[0] + 1) * D] * SCALING,
            bq_full[heads[1] * D : (heads[1] + 1) * D] * SCALING,
            bq2,
            bq2,
        ]
    ).astype(np.float32)[:, None]

    return {
        "xT": xT,
        "wqk": np.ascontiguousarray(wqk),
        "w2": np.ascontiguousarray(w2),
        "wv": np.ascontiguousarray(wv_arr),
        "wout": np.ascontiguousarray(wout),
        "bq": np.ascontiguousarray(bq),
    }


def kernel(x, in_proj_weight, in_proj_bias, out_proj_weight, out_proj_bias,
           _trace=False, _tmpdir=None):
    from concourse.bass_utils import run_bass_kernel_spmd

    x = np.asarray(x, dtype=np.float32)
    in_proj_weight = np.asarray(in_proj_weight, dtype=np.float32)
    in_proj_bias = np.asarray(in_proj_bias, dtype=np.float32)
    out_proj_weight = np.asarray(out_proj_weight, dtype=np.float32)
    out_proj_bias = np.asarray(out_proj_bias, dtype=np.float32)

    if "prog" not in _PROGRAM_CACHE:
        _PROGRAM_CACHE["prog"] = _build_program()
    nc = _PROGRAM_CACHE["prog"]

    in_maps = [
        _core_inputs(x, in_proj_weight, in_proj_bias, out_proj_weight, c)
        for c in range(N_CORES)
    ]
    res = run_bass_kernel_spmd(
        nc, in_maps, list(range(N_CORES)), trace=_trace, tmpdir=_tmpdir
    )
    _PROGRAM_CACHE["last_results"] = res

    # v-bias folds into the output bias: out += (bv_cat @ Wout^T + b_out)
    bv_cat = in_proj_bias[2 * E : 3 * E]
    bias_eff = out_proj_bias + out_proj_weight @ bv_cat

    out = np.empty((B, S, E), dtype=np.float32)
    for b in range(B):
        acc = res.results[b * CORES_PER_BATCH]["out_part"].copy()
        for c in range(b * CORES_PER_BATCH + 1, (b + 1) * CORES_PER_BATCH):
            acc += res.results[c]["out_part"]
        out[b] = acc.T + bias_eff[None, :]
    return (out, None)


# revision 14
# speedup vs baseline: 1.0614x; 1.0614x over previous
"""Trainium2 Bass kernel for nn_Attention (B=2, S=4096, E=768, H=12, D=64).

Sharding: 24 (batch, head) units over 8 cores -> 3 heads per core, one batch
per 4-core group. Each core computes QKV projections for its 3 heads, full
attention (scoresT layout [j, i]; softmax across partitions via a
ones-augmented V matmul), and its partial out-projection [E, S]. The host
sums the 4 partials per batch and adds the fused output bias.

Math notes:
 - k bias dropped (softmax is shift-invariant along the key axis).
 - q bias and the 1/sqrt(D) scaling folded into the q weights/bias.
 - v bias folded into the output bias on host (sum_j softmax = 1).
 - All matmul operands are fp16 (accumulation fp32 in PSUM). fp16 keeps
   LDWEIGHTS off the critical path (pull-ahead + FWL) and enables
   concurrent row-tiled score matmuls (two heads, or two key tiles of the
   third head via duplicated hi/lo q/k copies).

Schedule: ScalarE (exp over all S^2 scores) is the bottleneck engine, so
ic=0's attention is interleaved with the QKV projections to start exp work
within the first few microseconds; afterwards the per-i-chunk pipeline
keeps ScalarE 100% busy (measured) while PE runs scores/AV/out-proj
underneath it.
"""

import numpy as np

B = 2
S = 4096
E = 768
NHEADS = 12
D = 64
SCALING = float(D) ** -0.5
N_CORES = 8
HPC = 3  # heads per core
CORES_PER_BATCH = 4

_PROGRAM_CACHE = {}


def _build_program(s=S):
    import concourse.mybir as mybir
    import concourse.tile as tile
    from concourse import bacc

    f32 = mybir.dt.float32
    f16 = mybir.dt.float16
    Exp = mybir.ActivationFunctionType.Exp

    IC = 512  # i-chunk (query block, PSUM free dim)
    NIC = s // IC
    NJT = s // 128  # key tiles of 128
    NE = E // 128  # contraction tiles for the projections

    nc = bacc.Bacc(
        "TRN2", target_bir_lowering=False, debug=False, num_devices=N_CORES
    )

    xT_d = nc.dram_tensor("xT", [E, s], f16, kind="ExternalInput").ap()
    wqk_d = nc.dram_tensor("wqk", [E, 256], f16, kind="ExternalInput").ap()
    w2_d = nc.dram_tensor("w2", [E, 256], f16, kind="ExternalInput").ap()
    wv_d = nc.dram_tensor("wv", [E, 256], f16, kind="ExternalInput").ap()
    wout_d = nc.dram_tensor("wout", [D, HPC * E], f16, kind="ExternalInput").ap()
    bq_d = nc.dram_tensor("bq", [256, 1], f32, kind="ExternalInput").ap()
    out_d = nc.dram_tensor("out_part", [E, s], f32, kind="ExternalOutput").ap()

    with tile.TileContext(nc) as tc:
        with (
            tc.tile_pool(name="consts", bufs=1) as consts,
            tc.tile_pool(name="persist", bufs=1) as persist,
        ):
            # ---- weights ----
            def load_w(dram, cols, tag):
                t = consts.tile([128, NE * cols], f16, tag=tag)
                nc.sync.dma_start(
                    t[:].rearrange("p (e c) -> p e c", e=NE),
                    dram.rearrange("(e p) c -> p e c", p=128),
                )
                return t

            wqk_sb = load_w(wqk_d, 256, "wqk")
            w2_sb = load_w(w2_d, 256, "w2")
            wv_sb = load_w(wv_d, 256, "wv")
            wout_sb = consts.tile([D, HPC * E], f16, tag="wout")
            nc.sync.dma_start(wout_sb[:], wout_d)
            bqp_sb = consts.tile([128, 1], f32, tag="bqp")
            nc.sync.dma_start(bqp_sb[:], bq_d[0:128, :])
            bq2_sb = consts.tile([128, 1], f32, tag="bq2")
            nc.sync.dma_start(bq2_sb[:], bq_d[128:256, :])

            # ---- persistent activations ----
            # pair: h0 on partitions 0:64, h1 on 64:128
            # h2: duplicated on both partition halves (row-tiled dual stream)
            qTp = persist.tile([128, s], f16, tag="qTp")
            kTp = persist.tile([128, s], f16, tag="kTp")
            qT2 = persist.tile([128, s], f16, tag="qT2")
            kT2 = persist.tile([128, s], f16, tag="kT2")
            # v in natural [j, d] layout, 65-wide slots (col 64 = ones)
            v_aug = persist.tile([128, HPC * NJT * 65], f16, tag="vaug")
            vview = v_aug[:].rearrange("p (h j c) -> p h j c", h=HPC, c=65)
            ones_src = consts.tile([128, 1], f32, tag="ones")
            nc.vector.memset(ones_src[:], 1.0)
            nc.vector.tensor_copy(
                v_aug[:].rearrange("p (t c) -> p t c", c=65)[:, :, 64:65],
                ones_src[:, None, :].broadcast_to([128, HPC * NJT, 1]),
            )

            class Dual:
                """Slice-granular dual-row-tiled scores -> grouped exp -> AV,
                skewed by one exp group so PE never stalls on a fresh exp.

                One slice = one [128, 512] scores matmul on one partition
                half (row-tiled, so adjacent slices run concurrently on the
                PE). G slices share one PSUM tile and one exp op.
                """

                def __init__(self, sc_pool, e_pool, ic, kT, qT, G,
                             jt_of, rh_of, av_of, head_of):
                    self.sc_pool, self.e_pool = sc_pool, e_pool
                    self.isl = slice(ic * IC, (ic + 1) * IC)
                    self.kT, self.qT, self.G = kT, qT, G
                    self.jt_of, self.rh_of = jt_of, rh_of
                    self.av_of, self.head_of = av_of, head_of
                    self.s = 0
                    self.cur = None
                    self.pending = None  # (et, base_slice, count)

                def _avs(self, et, base, count):
                    for k in range(count):
                        sl = base + k
                        jt = self.jt_of(sl)
                        nc.tensor.matmul(
                            self.av_of(sl)[:],
                            vview[:, self.head_of(sl), jt, :],
                            et[:, k * 512 : (k + 1) * 512],
                            start=(jt == 0),
                            stop=(jt == NJT - 1),
                        )

                def _fire(self, count):
                    base = self.s - count
                    et = self.e_pool.tile([128, 512 * self.G], f16, tag="e")
                    nc.scalar.activation(
                        et[:, : 512 * count], self.cur[:, : 512 * count], Exp
                    )
                    if self.pending is not None:
                        self._avs(*self.pending)
                    self.pending = (et, base, count)
                    self.cur = None

                def emit(self):
                    pos = self.s % self.G
                    if pos == 0:
                        self.cur = self.sc_pool.tile(
                            [128, 512 * self.G], f32, tag="sc"
                        )
                    jt = self.jt_of(self.s)
                    rh = self.rh_of(self.s)
                    plo = rh * 64
                    nc.tensor.matmul(
                        self.cur[:, pos * 512 : (pos + 1) * 512],
                        self.kT[plo : plo + 64, jt * 128 : (jt + 1) * 128],
                        self.qT[plo : plo + 64, self.isl],
                        start=True,
                        stop=True,
                        tile_position=(plo, 0),
                    )
                    self.s += 1
                    if self.s % self.G == 0:
                        self._fire(self.G)

                def finish(self):
                    if self.s % self.G != 0:
                        self._fire(self.s % self.G)
                    if self.pending is not None:
                        self._avs(*self.pending)
                        self.pending = None

            with (
                tc.tile_pool(name="xt", bufs=NIC * NE) as xt_pool,
                tc.tile_pool(name="e_sb", bufs=4) as e_pool,
                tc.tile_pool(name="outt", bufs=2) as outT_pool,
                tc.tile_pool(name="bcast", bufs=2) as bcast_pool,
                tc.tile_pool(name="recip", bufs=2) as recip_pool,
                tc.tile_pool(name="op_out", bufs=3) as op_out_pool,
            ):
                def normalize(av, outT, h):
                    srow = recip_pool.tile([1, IC], f32, tag="srow")
                    nc.vector.tensor_copy(srow[:], av[64:65, :])
                    rc = recip_pool.tile([1, IC], f32, tag="recip")
                    nc.vector.reciprocal_approx_fast(rc[:], srow[:])
                    bc = bcast_pool.tile([64, IC], f32, tag="bcast")
                    nc.gpsimd.partition_broadcast(bc[:], rc[:], channels=64)
                    nc.vector.tensor_mul(
                        outT[:, h * IC : (h + 1) * IC], av[0:64, :], bc[:]
                    )

                # All xT loads up front (tiles stay resident; the DMA
                # engines run ahead of compute)
                xts = {}
                for cg in range(NIC):
                    for e in range(NE):
                        t = xt_pool.tile([128, IC], f16, tag="xt")
                        nc.sync.dma_start(
                            t[:],
                            xT_d[e * 128 : (e + 1) * 128,
                                 cg * IC : (cg + 1) * IC],
                        )
                        xts[(cg, e)] = t

                # ---- interleaved projections + ic=0 attention ----
                outT0 = outT_pool.tile([64, HPC * IC], f16, tag="outt")
                with (
                    tc.tile_pool(name="acc_i", bufs=4, space="PSUM") as acc_i,
                    tc.tile_pool(name="proj_ps", bufs=2, space="PSUM") as proj_ps,
                    tc.tile_pool(name="sc_i", bufs=1, space="PSUM") as sc_i,
                ):
                    av0 = acc_i.tile([65, IC], f32, tag="acc")
                    av1 = acc_i.tile([65, IC], f32, tag="acc")
                    av2 = acc_i.tile([65, IC], f32, tag="acc")
                    avs = (av0, av1)
                    pairE = Dual(sc_i, e_pool, 0, kTp, qTp, 2,
                                 jt_of=lambda s: s // 2,
                                 rh_of=lambda s: s % 2,
                                 av_of=lambda s: avs[s % 2],
                                 head_of=lambda s: s % 2)
                    h2E = Dual(sc_i, e_pool, 0, kT2, qT2, 2,
                               jt_of=lambda s: s,
                               rh_of=lambda s: s % 2,
                               av_of=lambda s: av2,
                               head_of=lambda s: 2)
                    for cg in range(NIC):
                        cs = slice(cg * IC, (cg + 1) * IC)
                        # q/k pair projections, then the third head's
                        # (two psum groups at a time; bufs=2 rotation)
                        for wsb, qdst, kdst, bias in (
                            (wqk_sb, qTp, kTp, bqp_sb),
                            (w2_sb, qT2, kT2, bq2_sb),
                        ):
                            qps = proj_ps.tile([128, IC], f32, tag="proj")
                            kps = proj_ps.tile([128, IC], f32, tag="proj")
                            for e in range(NE):
                                st = dict(start=(e == 0), stop=(e == NE - 1))
                                c0 = e * 256
                                xe = xts[(cg, e)][:]
                                nc.tensor.matmul(
                                    qps[:], wsb[:, c0 : c0 + 128], xe, **st
                                )
                                nc.tensor.matmul(
                                    kps[:], wsb[:, c0 + 128 : c0 + 256], xe, **st
                                )
                            nc.vector.tensor_scalar_add(
                                qdst[:, cs], qps[:], bias[:]
                            )
                            nc.vector.tensor_copy(kdst[:, cs], kps[:])
                        # v projections + ic0 attention slices, interleaved
                        for l in range(4):
                            jt = cg * 4 + l
                            vp = acc_i.tile([128, 256], f32, tag="acc")
                            for e in range(NE):
                                nc.tensor.matmul(
                                    vp[:],
                                    xts[(cg, e)][:, l * 128 : (l + 1) * 128],
                                    wv_sb[:, e * 256 : (e + 1) * 256],
                                    start=(e == 0),
                                    stop=(e == NE - 1),
                                )
                            nc.vector.tensor_copy(
                                vview[:, :, jt, 0:64],
                                vp[:, 0:192].rearrange("p (h d) -> p h d", h=HPC),
                            )
                            pairE.emit()
                            pairE.emit()
                            # with a single scores slot a group must fully
                            # fire before the other emitter allocates it
                            if l % 2 == 0:
                                h2E.emit()
                                h2E.emit()
                    pairE.finish()
                    normalize(av0, outT0, 0)
                    normalize(av1, outT0, 1)
                    h2E.finish()
                    normalize(av2, outT0, 2)
                pending = (outT0, 0)

                # ---- steady state: i-chunks 1..NIC-1 ----
                with (
                    tc.tile_pool(name="acc_s", bufs=2, space="PSUM") as acc_s,
                    tc.tile_pool(name="sc_ps", bufs=2, space="PSUM") as sc_s,
                ):
                    def outproj_groups(outT, ic):
                        isl = slice(ic * IC, (ic + 1) * IC)
                        for fb in range(E // 128):
                            op = acc_s.tile([128, IC], f32, tag="acc")
                            for h in range(HPC):
                                nc.tensor.matmul(
                                    op[:],
                                    wout_sb[
                                        :,
                                        h * E + fb * 128 : h * E + (fb + 1) * 128,
                                    ],
                                    outT[:, h * IC : (h + 1) * IC],
                                    start=(h == 0),
                                    stop=(h == HPC - 1),
                                )
                            ob = op_out_pool.tile([128, IC], f32, tag="ob")
                            nc.vector.tensor_copy(ob[:], op[:])
                            nc.sync.dma_start(
                                out_d[fb * 128 : (fb + 1) * 128, isl], ob[:]
                            )
                            yield

                    for ic in range(1, NIC):
                        outT = outT_pool.tile([64, HPC * IC], f16, tag="outt")
                        a0 = acc_s.tile([65, IC], f32, tag="acc")
                        a1 = acc_s.tile([65, IC], f32, tag="acc")
                        aa = (a0, a1)
                        d = Dual(sc_s, e_pool, ic, kTp, qTp, 3,
                                 jt_of=lambda s: s // 2,
                                 rh_of=lambda s: s % 2,
                                 av_of=lambda s, aa=aa: aa[s % 2],
                                 head_of=lambda s: s % 2)
                        for _ in range(2 * NJT):
                            d.emit()
                        d.finish()
                        normalize(a0, outT, 0)
                        normalize(a1, outT, 1)
                        # out-proj of the previous i-chunk is spread through
                        # the h2 section so ScalarE never drains its backlog
                        ops = outproj_groups(*pending)
                        a2 = acc_s.tile([65, IC], f32, tag="acc")
                        d2 = Dual(sc_s, e_pool, ic, kT2, qT2, 3,
                                  jt_of=lambda s: s,
                                  rh_of=lambda s: s % 2,
                                  av_of=lambda s, a2=a2: a2,
                                  head_of=lambda s: 2)
                        for g in range(NJT):
                            d2.emit()
                            if g % 5 == 4:
                                next(ops, None)
                        d2.finish()
                        for _ in ops:
                            pass
                        normalize(a2, outT, 2)
                        pending = (outT, ic)
                    for _ in outproj_groups(*pending):
                        pass

    nc.compile()
    return nc


def _core_inputs(x, in_proj_weight, in_proj_bias, out_proj_weight, core):
    """Host-side slicing for one core."""
    b = core // CORES_PER_BATCH
    h0 = HPC * (core % CORES_PER_BATCH)
    heads = [h0, h0 + 1, h0 + 2]

    wq = in_proj_weight[0:E]  # [E(out), E(in)]
    wk = in_proj_weight[E : 2 * E]
    wv = in_proj_weight[2 * E : 3 * E]
    bq_full = in_proj_bias[0:E]

    def head_wT(w, h):  # -> [E(in), D] = W_h.T
        return w[h * D : (h + 1) * D, :].T

    xT = np.ascontiguousarray(x[b].T).astype(np.float16)

    wqk = np.concatenate(
        [
            head_wT(wq, heads[0]) * SCALING,
            head_wT(wq, heads[1]) * SCALING,
            head_wT(wk, heads[0]),
            head_wT(wk, heads[1]),
        ],
        axis=1,
    ).astype(np.float16)
    wq2 = head_wT(wq, heads[2]) * SCALING
    wk2 = head_wT(wk, heads[2])
    w2 = np.concatenate([wq2, wq2, wk2, wk2], axis=1).astype(np.float16)
    wv_arr = np.concatenate(
        [head_wT(wv, h) for h in heads] + [np.zeros((E, D), np.float32)], axis=1
    ).astype(np.float16)
    wout = np.concatenate(
        [out_proj_weight[:, h * D : (h + 1) * D].T for h in heads], axis=1
    ).astype(np.float16)
    bq2 = bq_full[heads[2] * D : (heads[2] + 1) * D] * SCALING
    bq = np.concatenate(
        [
            bq_full[heads[0] * D : (heads[0] + 1) * D] * SCALING,
            bq_full[heads[1] * D : (heads[1] + 1) * D] * SCALING,
            bq2,
            bq2,
        ]
    ).astype(np.float32)[:, None]

    return {
        "xT": xT,
        "wqk": np.ascontiguousarray(wqk),
        "w2": np.ascontiguousarray(w2),
        "wv": np.ascontiguousarray(wv_arr),
        "wout": np.ascontiguousarray(wout),
        "bq": np.ascontiguousarray(bq),
    }


def kernel(x, in_proj_weight, in_proj_bias, out_proj_weight, out_proj_bias,
           _trace=False, _tmpdir=None):
    from concourse.bass_utils import run_bass_kernel_spmd

    x = np.asarray(x, dtype=np.float32)
    in_proj_weight = np.asarray(in_proj_weight, dtype=np.float32)
    in_proj_bias = np.asarray(in_proj_bias, dtype=np.float32)
    out_proj_weight = np.asarray(out_proj_weight, dtype=np.float32)
    out_proj_bias = np.asarray(out_proj_bias, dtype=np.float32)

    if "prog" not in _PROGRAM_CACHE:
        _PROGRAM_CACHE["prog"] = _build_program()
    nc = _PROGRAM_CACHE["prog"]

    in_maps = [
        _core_inputs(x, in_proj_weight, in_proj_bias, out_proj_weight, c)
        for c in range(N_CORES)
    ]
    res = run_bass_kernel_spmd(
        nc, in_maps, list(range(N_CORES)), trace=_trace, tmpdir=_tmpdir
    )
    _PROGRAM_CACHE["last_results"] = res

    # v-bias folds into the output bias: out += (bv_cat @ Wout^T + b_out)
    bv_cat = in_proj_bias[2 * E : 3 * E]
    bias_eff = out_proj_bias + out_proj_weight @ bv_cat

    out = np.empty((B, S, E), dtype=np.float32)
    for b in range(B):
        acc = res.results[b * CORES_PER_BATCH]["out_part"].copy()
        for c in range(b * CORES_PER_BATCH + 1, (b + 1) * CORES_PER_BATCH):
            acc += res.results[c]["out_part"]
        out[b] = acc.T + bias_eff[None, :]
    return (out, None)
